# revision 7
# baseline (speedup 1.0000x reference)
"""DiagonalLSTM Trainium2 kernel.

Reference computation (per batch element b):
  xs = skew(x)                               # (Cin, H, 2W-1), row r shifted right by r
  z_is = w_is @ xs + b_is                    # 1x1 conv -> 4*HID channels
  for t in 0..2W-2:                          # sequential scan over skewed width
      hs[o, r] = wss[o,c,0] h[c,r-1] + wss[o,c,1] h[c,r] + b_ss[o]
      z = z_is[:, :, t] + hs
      i, f, o_, g = sig, sig, sig, tanh of the 4 gate quarters
      c = f*c + i*g ; h = o_*tanh(c)
  out = unskew(h history)

Sharding: data-parallel over batch B=8 across the 8 NeuronCores (the t-scan is
inherently sequential; each core runs its own batch element's full scan).

Per-core layout (128 partitions = channels):
 - gates-on-partitions: each gate quarter is a (128 chan x 64 row) tile; matmuls
   are lhsT=(128c x 128o) weights, rhs=(128c x 64r) h / skewed-x slices.
 - z_is is NOT precomputed: per step it is one fp16 matmul per gate directly
   into the step's PSUM bank (start=True), emitted LOOKAHEAD steps early so the
   PE does it while waiting for the recurrent h. The recurrent taps accumulate
   on top (start=False).
 - skewed x lives in SBUF as a zero-padded fp16 buffer xskew[c, r*127 + t]
   (equivalently r*128 + w for the valid slots), so the step-t rhs is the
   stride-127 diagonal slice and out-of-band rows read exact zeros.
 - h is written as fp16 into a (128 x 65) rhs buffer whose col 0 is always 0,
   so tap0 (h[r-1]) and tap1 (h[r]) are the same buffer at offsets 0 and 1.
 - h history is stored fp32 directly in unskewed layout hist[c, r*64+w] via a
   stride-63 write of the in-band rows (t = r+w  =>  flat = r*63 + t).
"""

import sys

if "/opt/trn_rl_repo" not in sys.path:
    sys.path.insert(0, "/opt/trn_rl_repo")

import numpy as np

N_CORES = 8
HID = 128
CIN = 128
H = 64
W = 64
T = 2 * W - 1  # 127
LOOKAHEAD = 2

_PROGRAM_CACHE = {}


def _build_program(use_bias: bool):
    import concourse.bacc as bacc
    import concourse.tile as tile
    from concourse import mybir

    fp32 = mybir.dt.float32
    fp16 = mybir.dt.float16
    AFT = mybir.ActivationFunctionType
    ALU = mybir.AluOpType

    nc = bacc.Bacc("TRN2", debug=False, num_devices=N_CORES)
    x_d = nc.dram_tensor("x", [CIN, H * W], fp32, kind="ExternalInput")
    wis_d = nc.dram_tensor("wis", [CIN, 4 * HID], fp16, kind="ExternalInput")
    wss0_d = nc.dram_tensor("wss0", [HID, 4 * HID], fp16, kind="ExternalInput")
    wss1_d = nc.dram_tensor("wss1", [HID, 4 * HID], fp16, kind="ExternalInput")
    bias_d = nc.dram_tensor("bias", [HID, 4], fp32, kind="ExternalInput")
    out_d = nc.dram_tensor("out", [HID, H * W], fp32, kind="ExternalOutput")

    with tile.TileContext(nc) as tc:
        with (
            tc.tile_pool(name="persist", bufs=1) as pp,
            tc.tile_pool(name="gates", bufs=3) as gp,
            tc.tile_pool(name="ps_ifo", bufs=4, space="PSUM") as ppi,
            tc.tile_pool(name="ps_g", bufs=4, space="PSUM") as ppg,
        ):
            xbuf = pp.tile([128, H * W], fp32, tag="xbuf")
            xskew = pp.tile([128, 64 * 128], fp16, tag="xskew")
            wis_s = pp.tile([128, 512], fp16, tag="wis")
            wss0_s = pp.tile([128, 512], fp16, tag="wss0")
            wss1_s = pp.tile([128, 512], fp16, tag="wss1")
            bias_s = pp.tile([128, 4], fp32, tag="bias")
            rhs = [
                pp.tile([128, 65], fp16, tag=f"rhs{i}", name=f"rhs{i}")
                for i in range(2)
            ]
            cbuf = pp.tile([128, 64], fp32, tag="cbuf")
            hist = pp.tile([128, H * W], fp32, tag="hist")
            warm = pp.tile([128, 1], fp32, tag="warm")

            # --- prologue ---
            nc.gpsimd.dma_start(out=xbuf, in_=x_d.ap())
            nc.gpsimd.dma_start(out=wis_s, in_=wis_d.ap())
            nc.gpsimd.dma_start(out=wss0_s, in_=wss0_d.ap())
            nc.gpsimd.dma_start(out=wss1_s, in_=wss1_d.ap())
            nc.gpsimd.dma_start(out=bias_s, in_=bias_d.ap())

            # Pull the sigmoid/tanh ACT table load to the start (overlaps DMA).
            nc.vector.memset(warm, 0.0)
            nc.scalar.activation(warm, warm, AFT.Sigmoid)
            nc.scalar.activation(warm, warm, AFT.Tanh)

            nc.vector.memset(xskew, 0.0)
            nc.vector.memset(rhs[0], 0.0)
            nc.vector.memset(rhs[1], 0.0)
            nc.vector.memset(cbuf, 0.0)

            # skew-cast: xskew[c, r*128 + w] = fp16(x[c, r*64 + w])
            xsk3 = xskew.rearrange("p (r w) -> p r w", w=128)[:, :, 0:64]
            nc.vector.tensor_copy(out=xsk3, in_=xbuf.rearrange("p (r w) -> p r w", w=64))

            def diag_rhs(t):
                # (128, 64) fp16: col r = xs[:, r, t] (zeros out of band)
                return xskew[:, t : t + 63 * 127 + 1 : 127]

            pifo = [None] * T
            pg = [None] * T

            def emit_z(t):
                # start=True clears has_written for the WHOLE psum bank, so
                # exactly one matmul per bank-round carries it (the first),
                # and cross-region order is pinned with explicit deps.
                pifo[t] = ppi.tile([128, 192], fp32, tag="pifo", name=f"pifo{t}")
                pg[t] = ppg.tile([128, 64], fp32, tag="pg", name=f"pg{t}")
                r = diag_rhs(t)
                nc.tensor.matmul(
                    pg[t], lhsT=wis_s[:, 384:512], rhs=r,
                    start=True, stop=False, skip_group_check=True,
                )
                first = None
                for q in range(3):
                    mm = nc.tensor.matmul(
                        pifo[t][:, q * 64 : (q + 1) * 64],
                        lhsT=wis_s[:, q * 128 : (q + 1) * 128],
                        rhs=r,
                        start=(q == 0),
                        stop=False,
                        skip_group_check=True,
                    )
                    if q == 0:
                        first = mm
                    else:
                        tile.add_dep_helper(
                            mm.ins, first.ins, sync=False,
                            reason="bank-clear MM must run first",
                        )

            for t in range(LOOKAHEAD):
                emit_z(t)

            # --- the 127-step scan ---
            for t in range(T):
                if t + LOOKAHEAD < T:
                    emit_z(t + LOOKAHEAD)

                rbuf = rhs[t % 2]
                tap0 = rbuf[:, 0:64]
                tap1 = rbuf[:, 1:65]
                # g quarter first so its tanh can start while i/f/o matmuls run
                nc.tensor.matmul(pg[t], lhsT=wss0_s[:, 384:512], rhs=tap0, start=False, stop=False, skip_group_check=True)
                nc.tensor.matmul(pg[t], lhsT=wss1_s[:, 384:512], rhs=tap1, start=False, stop=True, skip_group_check=True)
                for q in range(3):
                    dst = pifo[t][:, q * 64 : (q + 1) * 64]
                    nc.tensor.matmul(dst, lhsT=wss0_s[:, q * 128 : (q + 1) * 128], rhs=tap0, start=False, stop=False, skip_group_check=True)
                    nc.tensor.matmul(dst, lhsT=wss1_s[:, q * 128 : (q + 1) * 128], rhs=tap1, start=False, stop=True, skip_group_check=True)

                g_s = gp.tile([128, 64], fp32, tag="g")
                sig = gp.tile([128, 192], fp32, tag="sig")
                if use_bias:
                    nc.scalar.activation(g_s, pg[t], AFT.Tanh, bias=bias_s[:, 3:4])
                    for q in range(3):
                        nc.scalar.activation(
                            sig[:, q * 64 : (q + 1) * 64],
                            pifo[t][:, q * 64 : (q + 1) * 64],
                            AFT.Sigmoid,
                            bias=bias_s[:, q : q + 1],
                        )
                else:
                    nc.scalar.activation(g_s, pg[t], AFT.Tanh)
                    nc.scalar.activation(sig, pifo[t], AFT.Sigmoid)

                t1 = gp.tile([128, 64], fp32, tag="t1")
                t2 = gp.tile([128, 64], fp32, tag="t2")
                nc.vector.tensor_mul(t1, sig[:, 0:64], g_s)
                nc.vector.tensor_mul(t2, sig[:, 64:128], cbuf)
                nc.vector.tensor_tensor(cbuf, t1, t2, ALU.add)

                tc_s = gp.tile([128, 64], fp32, tag="tc")
                nc.scalar.activation(tc_s, cbuf, AFT.Tanh)

                # h (fp16) into the next rhs buffer -- this is the serial chain
                nbuf = rhs[(t + 1) % 2]
                nc.vector.tensor_mul(nbuf[:, 1:65], sig[:, 128:192], tc_s)

                # h (fp32) into unskewed history, in-band rows only (off chain)
                r0 = 0 if t < W else t - (W - 1)
                r1 = t if t < W else W - 1
                cnt = r1 - r0 + 1
                base = r0 * 63 + t
                hview = (
                    hist[:, base : base + (cnt - 1) * 63 + 1 : 63]
                    if cnt > 1
                    else hist[:, base : base + 1]
                )
                nc.vector.tensor_mul(hview, sig[:, 128 + r0 : 128 + r0 + cnt], tc_s[:, r0 : r0 + cnt])

            # --- epilogue ---
            nc.gpsimd.dma_start(out=out_d.ap(), in_=hist)

    nc.compile()
    return nc


def _get_program(use_bias: bool):
    if use_bias not in _PROGRAM_CACHE:
        _PROGRAM_CACHE[use_bias] = _build_program(use_bias)
    return _PROGRAM_CACHE[use_bias]


def kernel(x, w_is, b_is, w_ss, b_ss, _trace=False, _trace_kwargs=None):
    from concourse.bass_utils import run_bass_kernel_spmd

    x = np.asarray(x, dtype=np.float32)
    w_is = np.asarray(w_is, dtype=np.float32)
    b_is = np.asarray(b_is, dtype=np.float32)
    w_ss = np.asarray(w_ss, dtype=np.float32)
    b_ss = np.asarray(b_ss, dtype=np.float32)
    B = x.shape[0]
    assert x.shape == (B, CIN, H, W), x.shape

    bias = (b_is + b_ss).astype(np.float32)  # (512,)
    use_bias = bool(np.any(bias != 0.0))
    nc = _get_program(use_bias)

    wis_h = np.ascontiguousarray(w_is.T.astype(np.float16))  # (128, 512)
    wss0_h = np.ascontiguousarray(w_ss[:, :, 0, 0].T.astype(np.float16))  # (128, 512)
    wss1_h = np.ascontiguousarray(w_ss[:, :, 1, 0].T.astype(np.float16))  # (128, 512)
    bias_h = np.ascontiguousarray(bias.reshape(4, HID).T.astype(np.float32))  # (128, 4)

    in_maps = []
    for b in range(N_CORES):
        xb = x[b % B].reshape(CIN, H * W)
        in_maps.append(
            {
                "x": np.ascontiguousarray(xb),
                "wis": wis_h,
                "wss0": wss0_h,
                "wss1": wss1_h,
                "bias": bias_h,
            }
        )

    res = run_bass_kernel_spmd(
        nc,
        in_maps,
        core_ids=list(range(N_CORES)),
        trace=_trace,
        **(_trace_kwargs or {}),
    )
    out = np.stack(
        [res.results[b]["out"].reshape(HID, H, W) for b in range(B)], axis=0
    ).astype(np.float32)
    if _trace:
        return out, res
    return out
